# revision 52
# baseline (speedup 1.0000x reference)
"""CrossAttention Trainium2 kernel (8 NeuronCores, SPMD).

Sharding: 8 cores = batch(2) x query-block(4 x 1024). Each core computes a
[1024, 1024] slice of the output; no cross-core communication.

Reference math (per core, M=1024 query tokens, Skv=1024, D=1024, H=16, hd=64):
  q = hs @ Wq ; k = enc @ Wk ; v = enc @ Wv
  per-head LN(q), LN(k) over hd; scores = LN(q) @ LN(k)^T / sqrt(hd)
  out = softmax(scores) @ v ; return out @ Wo
Host folds the LN mean-centering into Wq/Wk (exact), pre-transposes
activations to feature-major, and casts matmul operands to bf16.

Schedule notes:
- all selector matmuls (ssq, rb) are bf16, and rinv = Rsqrt(ssq/64+eps) is
  ONE ScalarE op, so P1 uses a single act-table set and the Exp set loads
  exactly once at the first attention exp.
- projections emit the per-head sum-of-squares matmul one d-block late, so
  the in-order PE queue never waits on the ACT-copy -> DVE-square chain.
- Q projects both 512-token chunks in one d-loop (16 MMs per d) so the ssq
  chain of block d hides fully under block d+1's matmuls.
- scores per (c,p,v) are one j-stacked PSUM tile [128, 2j, 512]: the two
  head matmuls (K=64) go to row groups (0,0)/(64,0) adjacently and run
  concurrently; exp is one N=1024 ACTIVATE over both banks.
- AV lags its exp by two v-tiles and the software pipeline rolls ACROSS
  p-boundaries (next p's scores are emitted before the previous p's AV
  epilogue), so the 128-exp ScalarE stream runs gap-free at its floor.
- single-matmul filler packets (p25 normalize, O-proj of chunk 0) are
  pumped one per v-iteration into the ~0.26us/slot PE slack under the exp
  stream; O-proj tiles enqueue per-p so normalize pieces interleave, and
  the queue is double-pumped near the end so the tail starts empty.
- tail O-proj alternates between the oacc slots and the dead score PSUM
  slots, with drains on the idle ScalarE, so tiles pipeline instead of
  waiting on each other's drain copies.
- the V projection for heads 8-15 is deferred out of P1 into front-loaded
  2-MM filler pieces (AV first touches those heads at head-pair p4, ~37us
  into the exp stream; staging lands on DVE, not the bottleneck ScalarE),
  which starts the exp stream ~16us earlier.
- input loads: ~0.5MB per dma_start across all three DMA-capable queues
  (sync/scalar/gpsimd), enct+wk first (they gate the K projection).
- output tensor is fp16 (halves drain DMA); host casts back to fp32.
"""

import numpy as np
import ml_dtypes
from contextlib import ExitStack

import concourse.bass as bass
import concourse.tile as tile
from concourse import bacc, mybir
from concourse.bass_utils import run_bass_kernel_spmd

BF = mybir.dt.bfloat16
F32 = mybir.dt.float32
F16 = mybir.dt.float16

D = 1024      # model dim
H = 16        # heads
HD = 64       # head dim
M = 1024      # query tokens per core
SKV = 1024    # kv tokens (one batch)
B = 2
SQ = 4096
NCORES = 8
LN_EPS = 1e-5
CH = 512      # query-token chunk

ACT_EXP = mybir.ActivationFunctionType.Exp
ACT_SQRT = mybir.ActivationFunctionType.Sqrt

_cache = {}


def _selector_constants():
    # sel16[d][p, j]: 1 if head j == 2d + p//64  (sum-over-head-partitions lhsT)
    sel16 = np.zeros((8, 128, H), np.float32)
    for d in range(8):
        for p in range(128):
            sel16[d, p, 2 * d + p // 64] = 1.0
    # selB[d][j, p]: 1 if head j == 2d + p//64  (broadcast-to-head-partitions)
    selB = np.transpose(sel16, (0, 2, 1)).copy()
    bf = ml_dtypes.bfloat16
    return sel16.astype(bf), selB.astype(bf)


def _emit(ctx: ExitStack, tc, t, has_bias_q, has_bias_k):
    nc = tc.nc

    persist = ctx.enter_context(tc.tile_pool(name="persist", bufs=1))

    # ---- persistent SBUF tensors (stacked [128, 8, 1024] layout) ----
    hst = persist.tile([128, 8, M], BF, tag="hst")       # hs^T  (feature-major)
    enct = persist.tile([128, 8, SKV], BF, tag="enct")   # enc^T (feature-major)
    wq = persist.tile([128, 8, D], BF, tag="wq")
    wk = persist.tile([128, 8, D], BF, tag="wk")
    wv = persist.tile([128, 8, D], BF, tag="wv")
    wo = persist.tile([128, 8, D], BF, tag="wo")
    qtln = persist.tile([128, 8, M], BF, tag="qtln")     # LN(q)^T feature-major
    ktln = persist.tile([128, 8, SKV], BF, tag="ktln")   # LN(k)^T feature-major
    vaug = persist.tile([128, 8, H, HD + 1], BF, tag="vaug")  # [kv, h, V|1]
    aout = persist.tile([128, 8, M], BF, tag="aout")     # attn out^T
    gq_sb = persist.tile([128, 8], F32, tag="gq_sb")
    gk_sb = persist.tile([128, 8], F32, tag="gk_sb")
    selg_sb = persist.tile([128, 8, H], BF, tag="selg_sb")
    selB_sb = persist.tile([16, 8, 128], BF, tag="selB_sb")
    rinv_q = persist.tile([16, M], BF, tag="rinv_q")    # 1/std per (head, tok)
    rinv_k = persist.tile([16, SKV], BF, tag="rinv_k")
    sums_sb = persist.tile([16, M], F32, tag="sums_sb")   # softmax sums
    inv_s = persist.tile([16, M], BF, tag="inv_s")      # 1/softmax-sum
    eps_sb = persist.tile([16, 1], F32, tag="eps_sb")
    nc.vector.memset(eps_sb[:, :], LN_EPS)
    nc.vector.memset(sums_sb[:, :], 1.0)  # recip_fast is undefined on junk
    nc.vector.memset(vaug[:, :, :, HD:HD + 1], 1.0)
    bq_sb = persist.tile([128, 8], F32, tag="bq_sb") if has_bias_q else None
    bk_sb = persist.tile([128, 8], F32, tag="bk_sb") if has_bias_k else None

    # ---- loads: ~0.5MB per dma_start (each costs ~0.7us of queue issue
    # time), two queues in parallel, ordered by need-time ----
    def load3(eng, dst, src, g0, g1):
        eng.dma_start(dst[:, g0:g1, :],
                      src[g0 * 128:g1 * 128, :].rearrange(
                          "(g p) d -> p g d", p=128))

    load3(nc.sync, enct, t["encT"], 0, 2)
    load3(nc.scalar, enct, t["encT"], 2, 4)
    load3(nc.gpsimd, wk, t["wk"], 0, 2)
    load3(nc.sync, enct, t["encT"], 4, 6)
    load3(nc.scalar, enct, t["encT"], 6, 8)
    load3(nc.gpsimd, wk, t["wk"], 2, 4)
    load3(nc.sync, wk, t["wk"], 4, 6)
    load3(nc.scalar, wk, t["wk"], 6, 8)
    nc.sync.dma_start(selg_sb[:, :, :],
                      t["sel16"].rearrange("d p j -> p d j"))
    nc.sync.dma_start(selB_sb[:, :, :], t["selB"].rearrange("d j p -> j d p"))
    nc.sync.dma_start(gq_sb[:, :], t["gq"].rearrange("(d p) -> p d", p=128))
    nc.sync.dma_start(gk_sb[:, :], t["gk"].rearrange("(d p) -> p d", p=128))
    if has_bias_q:
        nc.sync.dma_start(bq_sb[:, :], t["bq"].rearrange("(d p) -> p d", p=128))
    if has_bias_k:
        nc.sync.dma_start(bk_sb[:, :], t["bk"].rearrange("(d p) -> p d", p=128))
    for g in range(4):
        load3(nc.gpsimd, wv, t["wv"], 2 * g, 2 * g + 2)
    for g in range(4):
        load3(nc.sync, hst, t["hsT"], 2 * g, 2 * g + 2)
    for g in range(4):
        load3(nc.gpsimd, wq, t["wq"], 2 * g, 2 * g + 2)
    for g in range(2):
        load3(nc.gpsimd, wo, t["wo"], 4 * g, 4 * g + 4)

    def recip_fast(out_ap, in_ap):
        from concourse.dve_ops import (
            RECIP_APPROX_FAST_CONSTS,
            RECIPROCAL_APPROX_FAST,
        )
        c = RECIP_APPROX_FAST_CONSTS
        nc.vector._custom_dve(
            RECIPROCAL_APPROX_FAST, out=out_ap, in0=in_ap,
            s0=c["s0"], s1=c["s1"], imm2=c["imm2"],
        )

    sq_pool = ctx.enter_context(tc.tile_pool(name="sq_pool", bufs=6))
    rv_pool = ctx.enter_context(tc.tile_pool(name="rv_pool", bufs=2))

    # ---------------- projection helpers ----------------
    def proj_mms(ps_proj, ps_ssq, w_sb, x_sb, ln_sb, cg):
        """Emit projection matmuls + staging + per-head sum-of-squares for
        token chunks cg. Returns the ssq PSUM tiles (per chunk)."""
        ssqs = {}
        for c in cg:
            ssqs[c] = ps_ssq.tile([16, 512], F32, tag="ssq", name=f"ssq{c}")

        def emit_ssq(d, sqs):
            for c in cg:
                nc.tensor.matmul(
                    ssqs[c][:, :],
                    lhsT=selg_sb[:, d, :],
                    rhs=sqs[c][:, :],
                    start=(d == 0), stop=(d == 7),
                    skip_group_check=True,
                )

        pend = []
        for d in range(8):
            accs = {c: ps_proj.tile([128, 512], F32, tag="acc",
                                    name=f"acc{c}") for c in cg}
            for k in range(8):
                for c in cg:
                    nc.tensor.matmul(
                        accs[c][:, :],
                        lhsT=w_sb[:, k, d * 128:(d + 1) * 128],
                        rhs=x_sb[:, k, c * 512:(c + 1) * 512],
                        start=(k == 0), stop=(k == 7),
                    )
            # ssq lags TWO d-blocks: its ACT-copy -> DVE-square chain is
            # ~3us, longer than one 16-MM group at the start.
            if len(pend) == 2:
                emit_ssq(*pend.pop(0))
            sqs = {}
            for c in cg:
                acc = accs[c]
                nc.scalar.copy(ln_sb[:, d, c * 512:(c + 1) * 512], acc[:, :])
                sq = sq_pool.tile([128, 512], BF, name=f"sq{c}")
                nc.vector.tensor_mul(sq[:, :],
                                     ln_sb[:, d, c * 512:(c + 1) * 512],
                                     ln_sb[:, d, c * 512:(c + 1) * 512])
                sqs[c] = sq
            pend.append((d, sqs))
        for d_, sqs_ in pend:
            emit_ssq(d_, sqs_)
        return ssqs

    def proj_rinv(ssqs, rinv_sb):
        # rinv = (ssq/64 + eps)^(-1/2): ACT Sqrt (the only P1 table set)
        # then the fast DVE reciprocal, bf16 out for the rb matmuls
        for c, ssq in ssqs.items():
            rstd = rv_pool.tile([16, 512], F32, tag="rstd")
            nc.scalar.activation(
                rstd[:, :], ssq[:, :], ACT_SQRT,
                bias=eps_sb[:, :], scale=1.0 / HD,
            )
            recip_fast(rinv_sb[:, c * 512:(c + 1) * 512], rstd[:, :])

    def proj_fin_d(ps_rb, rb_tag, ln_sb, g_sb, b_sb, rinv_sb, c, d):
        # apply: ln = raw * g * rinv (+ b) for feature block d of chunk c
        rb = ps_rb.tile([128, 512], F32, tag=rb_tag)
        nc.tensor.matmul(
            rb[:, :],
            lhsT=selB_sb[:, d, :],
            rhs=rinv_sb[:, c * 512:(c + 1) * 512],
            start=True, stop=True,
        )
        dst = ln_sb[:, d, c * 512:(c + 1) * 512]
        nc.vector.scalar_tensor_tensor(
            out=dst,
            in0=dst,
            scalar=g_sb[:, d:d + 1],
            in1=rb[:, :],
            op0=mybir.AluOpType.mult,
            op1=mybir.AluOpType.mult,
        )
        if b_sb is not None:
            nc.vector.tensor_scalar_add(dst, dst, b_sb[:, d:d + 1])

    # ---------------- P1: projections, finalize overlapped ----------------
    p1 = ExitStack()
    ps_proj = p1.enter_context(tc.tile_pool(name="ps_proj", bufs=3,
                                            space="PSUM"))
    ps_ssq = p1.enter_context(tc.tile_pool(name="ps_ssq", bufs=2,
                                           space="PSUM"))
    ps_rb = p1.enter_context(tc.tile_pool(name="ps_rb", bufs=2, space="PSUM"))

    ssq_k = proj_mms(ps_proj, ps_ssq, wk, enct, ktln, [0, 1])
    proj_rinv(ssq_k, rinv_k)

    k_fin = [(c, d) for c in range(2) for d in range(8)]

    # V projection, chunk-0 heads only (heads 8-15 follow as attention
    # fillers: AV first touches them at head-pair p4, ~37us into the exp
    # stream), K finalize interleaved
    for tt in range(8):
        acc = ps_proj.tile([128, 512], F32, tag="acc", name="acc0")
        for k in range(8):
            nc.tensor.matmul(
                acc[:, :],
                lhsT=enct[:, k, tt * 128:(tt + 1) * 128],
                rhs=wv[:, k, 0:512],
                start=(k == 0), stop=(k == 7),
            )
        nc.scalar.copy(vaug[:, tt, 0:8, 0:HD],
                       acc[:, :].rearrange("p (h e) -> p h e", e=HD))
        if 2 <= tt < 6:
            for c, d in k_fin[(tt - 2) * 4:(tt - 1) * 4]:
                proj_fin_d(ps_rb, "rb", ktln, gk_sb, bk_sb, rinv_k, c, d)

    # Q projection, both chunks in one d-loop (16 MMs/d hide the ssq chain).
    # Each d-block gets its own single-shot ssq tile (other head rows land
    # zero) so rinv + LN-finalize for block d-2 emit inside the loop and the
    # whole pipeline stays at PE pace with no DVE-paced tail.
    def q_rinv_fin(d, sqs):
        rv = rv_pool.tile([16, 2, 512], BF, tag="rv")
        for c in range(2):
            ssq = ps_ssq.tile([16, 512], F32, tag="ssq", name=f"qssq{c}")
            nc.tensor.matmul(
                ssq[:, :], lhsT=selg_sb[:, d, :], rhs=sqs[c][:, :],
                start=True, stop=True,
            )
            rstd = rv_pool.tile([16, 512], F32, tag="rstd")
            nc.scalar.activation(rstd[:, :], ssq[:, :], ACT_SQRT,
                                 bias=eps_sb[:, :], scale=1.0 / HD)
            recip_fast(rv[:, c, :], rstd[:, :])
        for c in range(2):
            rb = ps_rb.tile([128, 512], F32, tag="rb")
            nc.tensor.matmul(
                rb[:, :], lhsT=selB_sb[:, d, :], rhs=rv[:, c, :],
                start=True, stop=True,
            )
            dst = qtln[:, d, c * 512:(c + 1) * 512]
            nc.vector.scalar_tensor_tensor(
                out=dst, in0=dst, scalar=gq_sb[:, d:d + 1], in1=rb[:, :],
                op0=mybir.AluOpType.mult, op1=mybir.AluOpType.mult,
            )
            if bq_sb is not None:
                nc.vector.tensor_scalar_add(dst, dst, bq_sb[:, d:d + 1])

    q_pend = []
    for d in range(8):
        accs = {c: ps_proj.tile([128, 512], F32, tag="acc",
                                name=f"acc{c}") for c in range(2)}
        for k in range(8):
            for c in range(2):
                nc.tensor.matmul(
                    accs[c][:, :],
                    lhsT=wq[:, k, d * 128:(d + 1) * 128],
                    rhs=hst[:, k, c * 512:(c + 1) * 512],
                    start=(k == 0), stop=(k == 7),
                )
        if len(q_pend) == 2:
            q_rinv_fin(*q_pend.pop(0))
        sqs = {}
        for c in range(2):
            nc.scalar.copy(qtln[:, d, c * 512:(c + 1) * 512], accs[c][:, :])
            sq = sq_pool.tile([128, 512], BF, name=f"sq{c}")
            nc.vector.tensor_mul(sq[:, :],
                                 qtln[:, d, c * 512:(c + 1) * 512],
                                 qtln[:, d, c * 512:(c + 1) * 512])
            sqs[c] = sq
        q_pend.append((d, sqs))
    for d_, sqs_ in q_pend:
        q_rinv_fin(d_, sqs_)
    p1.close()

    # ---------------- P2: attention ----------------
    at_pool = ctx.enter_context(tc.tile_pool(name="at_pool", bufs=4))
    srow_pool = ctx.enter_context(tc.tile_pool(name="srow_pool", bufs=2))
    out_pool = ctx.enter_context(tc.tile_pool(name="out_pool", bufs=2))
    p2 = ExitStack()
    ps_sc = p2.enter_context(tc.tile_pool(name="ps_sc", bufs=2, space="PSUM"))
    ps_av = p2.enter_context(tc.tile_pool(name="ps_av", bufs=2, space="PSUM"))
    # ps_out doubles as the rb pool for p25: its "oacc" slots are idle
    # during chunk 0 and rotate fast enough during chunk 1.
    ps_out = p2.enter_context(tc.tile_pool(name="ps_out", bufs=2,
                                           space="PSUM"))

    def p25_pieces(c, p, q):
        # refresh 1/sums for this chunk (rows of still-undrained heads are
        # junk, but selB picks only rows 2p/2p+1) and scale aout block p.
        # Two queue pieces: the second is DVE-only, so it costs no PE
        # filler budget.
        state = {}

        def piece_a():
            # DVE-only: costs no PE budget, and puts the reciprocal one
            # slot ahead of the rb matmul so the in-order PE queue never
            # waits on it
            recip_fast(inv_s[:, c * CH:(c + 1) * CH],
                       sums_sb[:, c * CH:(c + 1) * CH])

        def piece_b():
            rb = ps_out.tile([128, CH], F32, tag="oacc")
            nc.tensor.matmul(
                rb[:, :],
                lhsT=selB_sb[:, p, :],
                rhs=inv_s[:, c * CH:(c + 1) * CH],
                start=True, stop=True,
            )
            state["rb"] = rb

        def piece_c():
            sl = aout[:, p, c * CH:(c + 1) * CH]
            nc.vector.tensor_mul(sl, sl, state["rb"][:, :])

        q.append(piece_a)
        q.append(piece_b)
        q.append(piece_c)

    def oproj_drain(tt, accs, on_act=False):
        for cc in range(2):
            ot = out_pool.tile([128, 512], F16)
            if on_act:
                nc.scalar.copy(ot[:, :], accs[cc][:, :])
            else:
                nc.vector.tensor_copy(ot[:, :], accs[cc][:, :])
            nc.sync.dma_start(
                t["out"][tt * 128:(tt + 1) * 128,
                         cc * 512:(cc + 1) * 512],
                ot[:, :],
            )

    def oproj_tt(tt):
        if tt % 2 == 1:
            scp = ps_sc.tile([128, 2, CH], F32, tag="sc")
            accs = {cc: scp[:, cc, :] for cc in range(2)}
        else:
            accs = {cc: ps_out.tile([128, 512], F32, tag="oacc",
                                    name=f"oacc{cc}") for cc in range(2)}
        for k in range(8):
            for cc in range(2):
                nc.tensor.matmul(
                    accs[cc][:, :],
                    lhsT=aout[:, k, tt * 128:(tt + 1) * 128],
                    rhs=wo[:, k, cc * 512:(cc + 1) * 512],
                    start=(k == 0), stop=(k == 7),
                )
        oproj_drain(tt, accs, on_act=True)

    # Filler: small PE packets (~400ns) pumped one per v-iteration so the
    # PE tracks the ACT exp pace without head-of-line stalls.
    fillers = []

    def make_fillers_oproj(tt):
        # single-MM pieces: chunk-1's per-v-slot PE slack under the exp
        # stream is ~0.26us, which fits one matmul but not two
        state = {}
        for k in range(8):
            for cc in range(2):
                def piece(k=k, cc=cc):
                    if k == 0 and cc == 0:
                        state["accs"] = {
                            c2: ps_out.tile([128, 512], F32, tag="oacc",
                                            name=f"oacc{c2}")
                            for c2 in range(2)}
                    nc.tensor.matmul(
                        state["accs"][cc][:, :],
                        lhsT=aout[:, k, tt * 128:(tt + 1) * 128],
                        rhs=wo[:, k, cc * 512:(cc + 1) * 512],
                        start=(k == 0), stop=(k == 7),
                    )
                    if k == 7 and cc == 1:
                        oproj_drain(tt, state["accs"])
                fillers.append(piece)

    def pump_filler():
        if fillers:
            fillers.pop(0)()

    def emit_av(entry):
        cc, pp, avs_e, v, at = entry
        for j in range(2):
            nc.tensor.matmul(
                avs_e[j][:, :],
                lhsT=vaug[:, v, 2 * pp + j, :],
                rhs=at[:, j, :],
                start=(v == 0), stop=(v == 7),
                skip_group_check=True,
            )
        if v == 7:
            # drain: attn out + softmax sums (row HD of augmented AV).
            # Engines need 32-aligned partition bases, so stage the sum
            # row at partition 0 and DMA-scatter into head-row h.
            for j in range(2):
                h = 2 * pp + j
                av = avs_e[j]
                nc.vector.tensor_copy(
                    aout[j * 64:(j + 1) * 64, pp, cc * CH:(cc + 1) * CH],
                    av[0:HD, :])
                srow = srow_pool.tile([1, CH], F32)
                nc.vector.tensor_copy(srow[:, :], av[HD:HD + 1, :])
                nc.sync.dma_start(
                    sums_sb[h:h + 1, cc * CH:(cc + 1) * CH], srow[:, :])
            # sums for (cc, pp) are now in flight; normalize is safe
            p25_pieces(cc, pp, fillers)

    def make_fillers_vc1(tt):
        # 2-MM pieces; staging on DVE (ScalarE is the chunk-0 bottleneck)
        state = {}
        for kk in range(4):
            def piece(kk=kk, tt=tt):
                if kk == 0:
                    state["acc"] = ps_out.tile([128, 512], F32, tag="oacc",
                                               name="vacc")
                for k in (2 * kk, 2 * kk + 1):
                    nc.tensor.matmul(
                        state["acc"][:, :],
                        lhsT=enct[:, k, tt * 128:(tt + 1) * 128],
                        rhs=wv[:, k, 512:1024],
                        start=(k == 0), stop=(k == 7),
                    )
                if kk == 3:
                    nc.vector.tensor_copy(
                        vaug[:, tt, 8:16, 0:HD],
                        state["acc"][:, :].rearrange("p (h e) -> p h e",
                                                     e=HD))
            fillers.append(piece)

    for tt_ in range(8):
        make_fillers_vc1(tt_)

    # Rolling software pipeline ACROSS p-boundaries: the next p's first
    # scores are emitted before the previous p's AV epilogue, so the exp
    # stream never waits at a boundary.
    pend = []
    for c in range(M // CH):
        for p in range(8):
            if c == 1 and 1 <= p <= 4:
                make_fillers_oproj(p - 1)
            avs = {j: ps_av.tile([HD + 1, CH], F32, tag="av", name=f"av{j}")
                   for j in range(2)}
            for v in range(8):
                sc = ps_sc.tile([128, 2, CH], F32, tag="sc")
                for j in range(2):
                    # K=64: row-tile the two heads onto disjoint PE row
                    # groups, emitted adjacently so they run concurrently
                    nc.tensor.matmul(
                        sc[:, j, :],
                        lhsT=ktln[j * 64:(j + 1) * 64, p,
                                  v * 128:(v + 1) * 128],
                        rhs=qtln[j * 64:(j + 1) * 64, p,
                                 c * CH:(c + 1) * CH],
                        start=True, stop=True,
                        tile_position=(j * 64, 0),
                    )
                pump_filler()
                if c == 1 and p >= 5:
                    # drain the queue before the tail: a second piece per
                    # slot here beats paying for it serially after the loop
                    pump_filler()
                # AV lags TWO v-tiles behind its exp so the in-order PE
                # queue never reaches an AV whose exp is still running
                if len(pend) == 2:
                    emit_av(pend.pop(0))
                at = at_pool.tile([128, 2, CH], BF)
                nc.scalar.activation(
                    at[:, :, :], sc[:, :, :], ACT_EXP, scale=0.125,
                )
                pend.append((c, p, avs, v, at))
    while pend:
        emit_av(pend.pop(0))
    while fillers:
        pump_filler()

    # ---------------- P3: tail output projection (chunk 1 = tiles 4-7) ----
    for tt in range(4, 8):
        oproj_tt(tt)
    p2.close()


def _build(has_bias_q, has_bias_k):
    key = (has_bias_q, has_bias_k)
    if key in _cache:
        return _cache[key]
    nc = bacc.Bacc("TRN2", target_bir_lowering=False, debug=False,
                   num_devices=NCORES)
    t = {}

    def inp(name, shape, dt):
        t[name] = nc.dram_tensor(name, list(shape), dt, kind="ExternalInput").ap()

    inp("hsT", (D, M), BF)
    inp("encT", (D, SKV), BF)
    inp("wq", (D, D), BF)
    inp("wk", (D, D), BF)
    inp("wv", (D, D), BF)
    inp("wo", (D, D), BF)
    inp("gq", (D,), F32)
    inp("gk", (D,), F32)
    if has_bias_q:
        inp("bq", (D,), F32)
    if has_bias_k:
        inp("bk", (D,), F32)
    inp("sel16", (8, 128, H), BF)
    inp("selB", (8, H, 128), BF)
    t["out"] = nc.dram_tensor("out", [M, D], F16, kind="ExternalOutput").ap()

    with tile.TileContext(nc) as tc:
        with ExitStack() as ctx:
            _emit(ctx, tc, t, has_bias_q, has_bias_k)
    nc.finalize()
    _cache[key] = nc
    return nc


def _center_fold(W):
    # Fold per-head output-column mean removal into the weight matrix (exact).
    Wr = np.asarray(W, np.float32).reshape(D, H, HD)
    return (Wr - Wr.mean(axis=2, keepdims=True)).reshape(D, D)


def kernel(hidden_states, encoder_hidden_states, Wq, Wk, Wv, Wo,
           gq, bq, gk, bk, _trace=False):
    hs = np.asarray(hidden_states, np.float32)
    enc = np.asarray(encoder_hidden_states, np.float32)
    bq = np.asarray(bq, np.float32)
    bk = np.asarray(bk, np.float32)
    has_bias_q = bool(np.any(bq != 0))
    has_bias_k = bool(np.any(bk != 0))
    nc = _build(has_bias_q, has_bias_k)

    bf = ml_dtypes.bfloat16
    wq_bf = _center_fold(Wq).astype(bf)
    wk_bf = _center_fold(Wk).astype(bf)
    wv_bf = np.asarray(Wv, np.float32).astype(bf)
    wo_bf = np.asarray(Wo, np.float32).astype(bf)
    gq_rep = np.tile(np.asarray(gq, np.float32), H)
    gk_rep = np.tile(np.asarray(gk, np.float32), H)
    sel16, selB = _selector_constants()

    common = {
        "wq": wq_bf, "wk": wk_bf, "wv": wv_bf, "wo": wo_bf,
        "gq": gq_rep, "gk": gk_rep,
        "sel16": sel16, "selB": selB,
    }
    if has_bias_q:
        common["bq"] = np.tile(bq, H)
    if has_bias_k:
        common["bk"] = np.tile(bk, H)

    in_maps = []
    for core in range(NCORES):
        b, qb = divmod(core, 4)
        hsT = np.ascontiguousarray(
            hs[b, qb * M:(qb + 1) * M, :].T).astype(bf)
        encT = np.ascontiguousarray(enc[b].T).astype(bf)
        in_maps.append({**common, "hsT": hsT, "encT": encT})

    res = run_bass_kernel_spmd(nc, in_maps, list(range(NCORES)), trace=_trace)

    out = np.empty((B, SQ, D), np.float32)
    for core in range(NCORES):
        b, qb = divmod(core, 4)
        out[b, qb * M:(qb + 1) * M, :] = \
            res.results[core]["out"].astype(np.float32)
    kernel.last_exec_time_ns = res.exec_time_ns
    kernel.last_results = res
    return out
